# revision 1
# baseline (speedup 1.0000x reference)
"""Distributed GATv2 (2 layers + BN) Bass kernel for 8 trn2 NeuronCores.

Strategy: nodes partitioned by range across 8 cores (dst-ownership).
Each core:
  - computes BN1 stats partials -> AllReduce -> folds BN into Wl1/Wr1
  - computes xl1 = bn(x)@Wl1s for ALL nodes (bf16, local DRAM table)
  - computes xr1T (feat-major, + folded biases) for its own nodes
  - edge phase L1: per 128-edge subtile (degree-bucketed, dst-grouped):
      indirect-DMA gather of xl1[src] rows, feat-major z via PE
      (transpose-accumulate + identity-matmul of an AP-broadcast xr),
      LeakyReLU on ACT, logits via PE against block-diag att, exp with a
      global shift (softmax-invariant), transposed segment-sums numT/denT
      via one-hot matmuls, feat-major epilogue -> h1T (bf16)
  - one AllGather of h1T (+BN2 stat partials packed in 2 extra rows)
  - BN2 fold, xl2 table for all nodes, edge phase L2 (same scheme)
Output per core: outT [16, NODES_PAD] f32; host unpermutes/concats.
"""
import sys
import numpy as np

sys.path.insert(0, "/opt/trn_rl_repo")

import concourse.bass as bass          # noqa: E402
import concourse.bacc as bacc          # noqa: E402
import concourse.tile as tile          # noqa: E402
from concourse import mybir            # noqa: E402
from concourse.bass_utils import run_bass_kernel_spmd  # noqa: E402
from concourse.masks import make_identity  # noqa: E402

F32 = mybir.dt.float32
BF = mybir.dt.bfloat16
I32 = mybir.dt.int32
NPBF = mybir.dt.np(BF)

NCORES = 8
HEADS = 8
BN_EPS = 1e-5
NEG_SLOPE = 0.2
BUCKETS = (4, 8, 16, 32, 64, 128)


class Cfg:
    def __init__(self, n_nodes, in_dim, hid, out, m1, m2):
        self.N = n_nodes
        self.IN = in_dim
        self.HID = hid
        self.OUT = out
        self.F1 = HEADS * hid
        self.F2 = HEADS * out
        self.M1 = m1          # logit shift (softmax-invariant), layer 1
        self.M2 = m2
        self.NL = n_nodes // NCORES
        self.NT_PAD = ((n_nodes + 127) // 128) * 128
        self.KT = [min(128, in_dim), max(0, in_dim - 128)]  # K tiles for IN


def _schedule(cfg, deg_per_core):
    """Uniform-across-cores bucketed subtile schedule."""
    # counts per bucket per core
    t_b = {}
    for b in BUCKETS:
        lo = 0 if b == BUCKETS[0] else BUCKETS[BUCKETS.index(b) - 1]
        cnt = max(int(((d > lo) & (d <= b)).sum()) for d in deg_per_core)
        nps = 128 // b
        t_b[b] = ((cnt + nps - 1) // nps) * nps
    tot = sum(t_b.values())
    pad = (-tot) % 128
    # tot is a multiple of 4 (every t_b divisible by its nps>=1; smallest nps=1
    # for b=128 so not guaranteed -- fix by padding b=128 count to mult of 4)
    # simpler: bump b=BUCKETS[-1] (nps=1) then b=BUCKETS[0] (nps=32):
    b_last = BUCKETS[-1]
    add_last = pad % 32
    t_b[b_last] += add_last
    pad -= add_last
    t_b[BUCKETS[0]] += pad
    nodes_pad = sum(t_b.values())
    assert nodes_pad % 128 == 0
    subtiles = []   # (bucket, node_start)
    pos = 0
    for b in BUCKETS:
        nps = 128 // b
        assert t_b[b] % nps == 0
        for k in range(t_b[b] // nps):
            subtiles.append((b, pos))
            pos += nps
    assert pos == nodes_pad
    return t_b, nodes_pad, subtiles


def _preprocess(cfg, x, edge_index, W):
    N, NL = cfg.N, cfg.NL
    src = np.concatenate([edge_index[0], np.arange(N, dtype=np.int32)])
    dst = np.concatenate([edge_index[1], np.arange(N, dtype=np.int32)])
    order = np.argsort(dst, kind="stable")
    src, dst = src[order], dst[order]
    deg = np.bincount(dst, minlength=N)
    starts = np.zeros(N + 1, np.int64)
    np.cumsum(deg, out=starts[1:])
    deg_pc = [deg[c * NL:(c + 1) * NL] for c in range(NCORES)]
    t_b, NODES_PAD, subtiles = _schedule(cfg, deg_pc)
    NSUB = len(subtiles)
    NG = NODES_PAD // 128

    # per-core node processing order (bucket-sorted) and dummy slots (-1)
    proc = np.full((NCORES, NODES_PAD), -1, np.int64)   # proc pos -> local node
    ppos = np.full((NCORES, NL), -1, np.int64)          # local node -> proc pos
    for c in range(NCORES):
        pos = 0
        d = deg_pc[c]
        for b in BUCKETS:
            lo = 0 if b == BUCKETS[0] else BUCKETS[BUCKETS.index(b) - 1]
            ids = np.nonzero((d > lo) & (d <= b))[0]
            proc[c, pos:pos + len(ids)] = ids
            ppos[c, ids] = pos + np.arange(len(ids))
            pos += t_b[b]
    # storage index of a global node: core*NODES_PAD + ppos
    store = np.empty(N, np.int64)
    for c in range(NCORES):
        store[c * NL:(c + 1) * NL] = c * NODES_PAD + ppos[c]

    esrc1 = np.zeros((NCORES, 128, NSUB), np.int32)
    esrc2 = np.zeros((NCORES, 128, NSUB), np.int32)
    edsts = np.full((NCORES, 128, NSUB), -1.0, np.float32)
    for c in range(NCORES):
        for s, (b, nstart) in enumerate(subtiles):
            nps = 128 // b
            for slot in range(nps):
                v = proc[c, nstart + slot]
                if v < 0:
                    # dummy keep-alive edge so den > 0
                    edsts[c, slot * b, s] = slot
                    continue
                gv = c * NL + int(v)
                e0 = starts[gv]
                dv = int(deg[gv])
                p0 = slot * b
                esrc1[c, p0:p0 + dv, s] = src[e0:e0 + dv]
                esrc2[c, p0:p0 + dv, s] = store[src[e0:e0 + dv]]
                edsts[c, p0:p0 + dv, s] = slot

    # inputs per core
    xT = np.zeros((cfg.IN, cfg.NT_PAD), NPBF)
    xT[:, :N] = x.T.astype(NPBF)
    in_maps = []
    A1 = np.zeros((cfg.F1, HEADS), np.float32)
    for h in range(HEADS):
        A1[h * cfg.HID:(h + 1) * cfg.HID, h] = W["att1"][h]
    A2 = np.zeros((cfg.F2, HEADS), np.float32)
    for h in range(HEADS):
        A2[h * cfg.OUT:(h + 1) * cfg.OUT, h] = W["att2"][h]
    for c in range(NCORES):
        xTo = np.zeros((cfg.IN, NODES_PAD), NPBF)
        sel = proc[c] >= 0
        xTo[:, sel] = x[c * NL + proc[c][sel]].T.astype(NPBF)
        mask32 = np.zeros((cfg.HID, NODES_PAD), NPBF)
        mask32[:, sel] = 1.0
        mean1m = np.zeros((128, cfg.HID), np.float32)
        mean1m[np.arange(128), np.arange(128) % cfg.HID] = 0.125
        mean2m = np.zeros((128, cfg.OUT), np.float32)
        mean2m[np.arange(128), np.arange(128) % cfg.OUT] = 0.125
        e1m = np.zeros((8, cfg.F1), np.float32)
        e1m[np.arange(cfg.F1) // cfg.HID, np.arange(cfg.F1)] = 1.0
        e2m = np.zeros((8, cfg.F2), np.float32)
        e2m[np.arange(cfg.F2) // cfg.OUT, np.arange(cfg.F2)] = 1.0
        in_maps.append({
            "xT": xT, "xTo": xTo, "mask32": mask32,
            "mean1m": mean1m, "mean2m": mean2m, "e1m": e1m, "e2m": e2m,
            "esrc1": np.ascontiguousarray(esrc1[c]),
            "esrc2": np.ascontiguousarray(esrc2[c]),
            "edsts": np.ascontiguousarray(edsts[c]),
            "Wl1m": W["Wl1"].reshape(cfg.IN, HEADS, cfg.HID).mean(1).astype(np.float32),
            "Wl2m": W["Wl2"].reshape(cfg.HID, HEADS, cfg.OUT).mean(1).astype(np.float32),
            "Wl1": W["Wl1"].astype(np.float32),
            "Wr1": W["Wr1"].astype(np.float32),
            "Wl2": W["Wl2"].astype(np.float32),
            "Wr2": W["Wr2"].astype(np.float32),
            "A1": A1, "A2": A2,
            "gb1": np.stack([W["gamma1"], W["beta1"]], 1).astype(np.float32),
            "gb2": np.stack([W["gamma2"], W["beta2"]], 1).astype(np.float32),
            "b1c": W["b1"].reshape(-1, 1).astype(np.float32),
            "b2c": W["b2"].reshape(-1, 1).astype(np.float32),
        })
    meta = dict(NODES_PAD=NODES_PAD, NSUB=NSUB, NG=NG, subtiles=subtiles,
                proc=proc, in_maps=in_maps)
    return meta


def _build(cfg, meta):
    NODES_PAD, NSUB, NG = meta["NODES_PAD"], meta["NSUB"], meta["NG"]
    subtiles = meta["subtiles"]
    IN, F1, F2, HID, OUT = cfg.IN, cfg.F1, cfg.F2, cfg.HID, cfg.OUT
    K0, K1 = cfg.KT
    NTP = cfg.NT_PAD
    C1, C2 = HID, OUT
    n_xl1_tiles = NTP // 128
    n_xl2_tiles = NCORES * NODES_PAD // 128
    RECIP_N = 1.0 / cfg.N

    nc = bacc.Bacc("TRN2", target_bir_lowering=False, debug=False,
                   num_devices=NCORES)
    din = {}
    for name, shape, dt in [
            ("xT", [IN, NTP], BF), ("xTo", [IN, NODES_PAD], BF),
            ("mask32", [HID, NODES_PAD], BF),
            ("esrc1", [128, NSUB], I32), ("esrc2", [128, NSUB], I32),
            ("edsts", [128, NSUB], F32),
            ("Wl1", [IN, F1], F32), ("Wr1", [IN, F1], F32),
            ("Wl2", [HID, F2], F32), ("Wr2", [HID, F2], F32),
            ("A1", [F1, HEADS], F32), ("A2", [F2, HEADS], F32),
            ("gb1", [IN, 2], F32), ("gb2", [HID, 2], F32),
            ("b1c", [HID, 1], F32), ("b2c", [OUT, 1], F32),
            ("mean1m", [128, HID], F32), ("mean2m", [128, OUT], F32),
            ("Wl1m", [IN, HID], F32), ("Wl2m", [HID, OUT], F32),
            ("e1m", [8, F1], F32), ("e2m", [8, F2], F32)]:
        din[name] = nc.dram_tensor(name, shape, dt, kind="ExternalInput")
    outT = nc.dram_tensor("outT", [OUT, NODES_PAD], F32, kind="ExternalOutput")

    xl1_full = nc.dram_tensor("xl1_full", [NTP, F1], BF)
    xl2_full = nc.dram_tensor("xl2_full", [NCORES * NODES_PAD, F2], BF)
    st1_in = nc.dram_tensor("st1_in", [IN, 2], F32)
    bl1_d = nc.dram_tensor("bl1_d", [1, HID], F32)
    bl2_d = nc.dram_tensor("bl2_d", [1, OUT], F32)
    st1_out = nc.dram_tensor("st1_out", [IN, 2], F32)
    ag_in = nc.dram_tensor("ag_in", [HID + 2, NODES_PAD], BF)
    ag_out = nc.dram_tensor("ag_out", [NCORES * (HID + 2), NODES_PAD], BF,
                            addr_space="Shared")

    import contextlib
    with tile.TileContext(nc) as tc:
        ctx = contextlib.ExitStack()
        with ctx:
            cpool = ctx.enter_context(tc.tile_pool(name="const", bufs=1))
            rpool = ctx.enter_context(tc.tile_pool(name="resident", bufs=1))

            # ---------- constants ----------
            ident = cpool.tile([128, 128], BF)
            make_identity(nc, ident[:])
            ones_row = cpool.tile([1, 128], BF)
            nc.vector.memset(ones_row[:], 1.0)
            epsb = cpool.tile([128, 1], F32, tag="epsb")
            nc.vector.memset(epsb[:], BN_EPS)
            msh1 = cpool.tile([128, 1], F32, tag="msh1")
            nc.vector.memset(msh1[:], -float(cfg.M1))
            msh2 = cpool.tile([128, 1], F32, tag="msh2")
            nc.vector.memset(msh2[:], -float(cfg.M2))
            alph = cpool.tile([128, 1], F32, tag="alph")
            nc.vector.memset(alph[:], NEG_SLOPE)
            nslope = cpool.tile([128, 1], F32, tag="nslope")
            nc.vector.memset(nslope[:], -(1.0 - NEG_SLOPE))
            io_b = {}
            for b in set(bb for bb, _ in subtiles):
                nps = 128 // b
                it = cpool.tile([128, nps], I32, tag=f"io{b}i")
                nc.gpsimd.iota(it[:], [[1, nps]], channel_multiplier=0)
                ft = cpool.tile([128, nps], F32, tag=f"io{b}f")
                nc.vector.tensor_copy(out=ft[:], in_=it[:])
                io_b[b] = ft
            def const_bf(name, shape, tagn):
                tf = cpool.tile(shape, F32, tag=tagn + "f", name=tagn + "f")
                nc.sync.dma_start(out=tf[:], in_=din[name].ap())
                tb = cpool.tile(shape, BF, tag=tagn, name=tagn)
                nc.vector.tensor_copy(out=tb[:], in_=tf[:])
                return tb
            mean1 = const_bf("mean1m", [128, C1], "mean1")
            mean2 = const_bf("mean2m", [128, C2], "mean2")
            e1full = const_bf("e1m", [8, F1], "e1m")
            e2full = const_bf("e2m", [8, F2], "e2m")
            e1h = [e1full[:, h * 128:(h + 1) * 128] for h in range(F1 // 128)]
            e2h = [e2full[:, h * 128:(h + 1) * 128] for h in range(F2 // 128)]

            # index/dst tables resident
            esrc1_sb = rpool.tile([128, NSUB], I32)
            nc.sync.dma_start(out=esrc1_sb[:], in_=din["esrc1"].ap())
            esrc2_sb = rpool.tile([128, NSUB], I32)
            nc.sync.dma_start(out=esrc2_sb[:], in_=din["esrc2"].ap())
            edsts_sb = rpool.tile([128, NSUB], F32)
            nc.sync.dma_start(out=edsts_sb[:], in_=din["edsts"].ap())
            # A1/A2 bf16
            a1_sb = []
            for h in range(F1 // 128):
                t = rpool.tile([128, HEADS], BF, tag=f"a1_{h}")
                tf = rpool.tile([128, HEADS], F32, tag=f"a1f_{h}")
                nc.sync.dma_start(out=tf[:], in_=din["A1"].ap()[h * 128:(h + 1) * 128, :])
                nc.vector.tensor_copy(out=t[:], in_=tf[:])
                a1_sb.append(t)
            a2_sb = []
            for h in range(F2 // 128):
                t = rpool.tile([128, HEADS], BF, tag=f"a2_{h}")
                tf = rpool.tile([128, HEADS], F32, tag=f"a2f_{h}")
                nc.sync.dma_start(out=tf[:], in_=din["A2"].ap()[h * 128:(h + 1) * 128, :])
                nc.vector.tensor_copy(out=t[:], in_=tf[:])
                a2_sb.append(t)
            b1c_sb = rpool.tile([HID, 1], F32)
            nc.sync.dma_start(out=b1c_sb[:], in_=din["b1c"].ap())
            b2c_sb = rpool.tile([OUT, 1], F32)
            nc.sync.dma_start(out=b2c_sb[:], in_=din["b2c"].ap())
            mask32_sb = rpool.tile([HID, NODES_PAD], BF)
            nc.sync.dma_start(out=mask32_sb[:], in_=din["mask32"].ap())

            # xTo resident tiles (feat-major own nodes)
            kt_sizes = [K0] + ([K1] if K1 else [])
            xTo_sb = []
            for ki, ks in enumerate(kt_sizes):
                t = rpool.tile([ks, NODES_PAD], BF, tag=f"xTo{ki}")
                nc.sync.dma_start(out=t[:], in_=din["xTo"].ap()[ki * 128:ki * 128 + ks, :])
                xTo_sb.append(t)

            # ---------- phase A: BN1 stats + AllReduce ----------
            st_t = []
            with tc.tile_pool(name="pA", bufs=2) as pa, \
                 tc.tile_pool(name="pAs", bufs=1) as pas:
                for ki, ks in enumerate(kt_sizes):
                    st = pa.tile([ks, 2], F32, tag="st")
                    nc.vector.tensor_reduce(out=st[:, 0:1], in_=xTo_sb[ki][:],
                                            axis=mybir.AxisListType.X,
                                            op=mybir.AluOpType.add)
                    scr = pas.tile([ks, NODES_PAD], BF, tag="scr")
                    nc.scalar.activation(out=scr[:], in_=xTo_sb[ki][:],
                                         func=mybir.ActivationFunctionType.Square,
                                         accum_out=st[:, 1:2])
                    nc.sync.dma_start(out=st1_in.ap()[ki * 128:ki * 128 + ks, :],
                                      in_=st[:])
                    st_t.append(st)
            nc.gpsimd.collective_compute(
                "AllReduce", mybir.AluOpType.add,
                ins=[st1_in.ap()], outs=[st1_out.ap()],
                replica_groups=[list(range(NCORES))])

            # fold stats -> s1, t1 (per K tile) and scaled weights
            s1_t, t1_t = [], []
            wl1s, wr1s = [], []
            wl1f, wr1f = [], []
            with tc.tile_pool(name="pB", bufs=1) as pb:
                for ki, ks in enumerate(kt_sizes):
                    stg = pb.tile([ks, 2], F32, tag=f"stg{ki}")
                    nc.sync.dma_start(out=stg[:], in_=st1_out.ap()[ki * 128:ki * 128 + ks, :])
                    gb = pb.tile([ks, 2], F32, tag=f"gb{ki}")
                    nc.sync.dma_start(out=gb[:], in_=din["gb1"].ap()[ki * 128:ki * 128 + ks, :])
                    mean = pb.tile([ks, 1], F32, tag=f"mean{ki}")
                    nc.vector.tensor_scalar(out=mean[:], in0=stg[:, 0:1],
                                            scalar1=RECIP_N, scalar2=None,
                                            op0=mybir.AluOpType.mult)
                    q = pb.tile([ks, 1], F32, tag=f"q{ki}")
                    nc.vector.tensor_scalar(out=q[:], in0=stg[:, 1:2],
                                            scalar1=RECIP_N, scalar2=None,
                                            op0=mybir.AluOpType.mult)
                    m2 = pb.tile([ks, 1], F32, tag=f"m2{ki}")
                    nc.vector.tensor_tensor(out=m2[:], in0=mean[:], in1=mean[:],
                                            op=mybir.AluOpType.mult)
                    var = pb.tile([ks, 1], F32, tag=f"var{ki}")
                    nc.vector.tensor_tensor(out=var[:], in0=q[:], in1=m2[:],
                                            op=mybir.AluOpType.subtract)
                    sd = pb.tile([ks, 1], F32, tag=f"sd{ki}")
                    nc.scalar.activation(out=sd[:], in_=var[:],
                                         func=mybir.ActivationFunctionType.Sqrt,
                                         bias=epsb[:ks, :1])
                    rstd = pb.tile([ks, 1], F32, tag=f"rstd{ki}")
                    nc.vector.reciprocal(rstd[:], sd[:])
                    s1 = pb.tile([ks, 1], F32, tag=f"s1{ki}")
                    nc.vector.tensor_tensor(out=s1[:], in0=gb[:, 0:1], in1=rstd[:],
                                            op=mybir.AluOpType.mult)
                    ms = pb.tile([ks, 1], F32, tag=f"ms{ki}")
                    nc.vector.tensor_tensor(out=ms[:], in0=mean[:], in1=s1[:],
                                            op=mybir.AluOpType.mult)
                    t1 = pb.tile([ks, 1], F32, tag=f"t1{ki}")
                    nc.vector.tensor_tensor(out=t1[:], in0=gb[:, 1:2], in1=ms[:],
                                            op=mybir.AluOpType.subtract)
                    s1_t.append(s1)
                    t1_t.append(t1)
                    for wname, lst, lstf in (("Wl1", wl1s, wl1f), ("Wr1", wr1s, wr1f)):
                        wf = rpool.tile([ks, F1], F32, tag=f"{wname}f{ki}")
                        nc.sync.dma_start(out=wf[:], in_=din[wname].ap()[ki * 128:ki * 128 + ks, :])
                        ws = rpool.tile([ks, F1], BF, tag=f"{wname}s{ki}")
                        nc.vector.tensor_scalar(out=ws[:], in0=wf[:],
                                                scalar1=s1[:, :1], scalar2=None,
                                                op0=mybir.AluOpType.mult)
                        lst.append(ws)
                        lstf.append(wf)
                # bias12 = t1 @ (Wl1 + Wr1)  [1, F1]
                t1b = []
                for ki, ks in enumerate(kt_sizes):
                    tb = pb.tile([ks, 1], BF, tag=f"t1b{ki}")
                    nc.vector.tensor_copy(out=tb[:], in_=t1_t[ki][:])
                    t1b.append(tb)
                with tc.tile_pool(name="pBp", bufs=1, space="PSUM") as pbp:
                    bps = pbp.tile([1, F1], F32, space="PSUM")
                    first = True
                    for ki, ks in enumerate(kt_sizes):
                        for wf in (wl1f[ki], wr1f[ki]):
                            wb = pb.tile([ks, F1], BF, tag=f"wb{ki}")
                            nc.vector.tensor_copy(out=wb[:], in_=wf[:])
                            nc.tensor.matmul(out=bps[:], lhsT=t1b[ki][:], rhs=wb[:],
                                             start=first, stop=(ki == len(kt_sizes) - 1 and wf is wr1f[ki]))
                            first = False
                    bias12 = rpool.tile([1, F1], BF)
                    nc.vector.tensor_copy(out=bias12[:], in_=bps[:])
                with tc.tile_pool(name="pBq", bufs=1, space="PSUM") as pbq:
                    blp = pbq.tile([1, HID], F32, space="PSUM")
                    for ki, ks in enumerate(kt_sizes):
                        wmf = pb.tile([ks, HID], F32, tag=f"wmf{ki}",
                                      name=f"wmf{ki}")
                        nc.sync.dma_start(
                            out=wmf[:],
                            in_=din["Wl1m"].ap()[ki * 128:ki * 128 + ks, :])
                        wmb = pb.tile([ks, HID], BF, tag=f"wmb{ki}",
                                      name=f"wmb{ki}")
                        nc.vector.tensor_copy(out=wmb[:], in_=wmf[:])
                        nc.tensor.matmul(out=blp[:], lhsT=t1b[ki][:], rhs=wmb[:],
                                         start=(ki == 0),
                                         stop=(ki == len(kt_sizes) - 1))
                    blr = pb.tile([1, HID], F32, tag="blr")
                    nc.vector.tensor_copy(out=blr[:], in_=blp[:])
                    nc.sync.dma_start(out=bl1_d.ap(), in_=blr[:])
                blc = rpool.tile([HID, 1], F32)
                nc.sync.dma_start(out=blc[:], in_=bl1_d.ap())
                bias1t = rpool.tile([HID, 1], F32)
                nc.vector.tensor_tensor(out=bias1t[:], in0=blc[:], in1=b1c_sb[:],
                                        op=mybir.AluOpType.add)

            # ---------- phase A2: xr1T resident (feat-major, own nodes) ----------
            xr1T = []
            with tc.tile_pool(name="pC", bufs=2, space="PSUM") as pc, \
                 tc.tile_pool(name="pCs", bufs=2) as pcs:
                for g in range(NG):
                    halves = []
                    for h in range(F1 // 128):
                        ps = pc.tile([128, 128], F32, space="PSUM", tag="xr1p")
                        for ki, ks in enumerate(kt_sizes):
                            nc.tensor.matmul(
                                out=ps[:],
                                lhsT=wr1s[ki][:, h * 128:(h + 1) * 128],
                                rhs=xTo_sb[ki][:, g * 128:(g + 1) * 128],
                                start=(ki == 0), stop=False)
                        nc.tensor.matmul(out=ps[:],
                                         lhsT=bias12[:, h * 128:(h + 1) * 128],
                                         rhs=ones_row[:],
                                         start=False, stop=True)
                        t = rpool.tile([128, 128], BF, tag=f"xr1T_{g}_{h}")
                        eng = nc.vector if (g + h) % 2 == 0 else nc.scalar
                        if eng is nc.vector:
                            nc.vector.tensor_copy(out=t[:], in_=ps[:])
                        else:
                            nc.scalar.copy(out=t[:], in_=ps[:])
                        halves.append(t)
                    xr1T.append(halves)

            # ---------- phase A3: xl1_full table ----------
            with tc.tile_pool(name="pD", bufs=3) as pd, \
                 tc.tile_pool(name="pDp", bufs=2, space="PSUM") as pdp:
                for t in range(n_xl1_tiles):
                    lhs = []
                    for ki, ks in enumerate(kt_sizes):
                        lt = pd.tile([ks, 128], BF, tag=f"xl1l{ki}")
                        nc.sync.dma_start(
                            out=lt[:],
                            in_=din["xT"].ap()[ki * 128:ki * 128 + ks,
                                               t * 128:(t + 1) * 128])
                        lhs.append(lt)
                    ps = pdp.tile([128, F1], F32, space="PSUM", tag="xl1p")
                    for ki, ks in enumerate(kt_sizes):
                        nc.tensor.matmul(out=ps[:], lhsT=lhs[ki][:], rhs=wl1s[ki][:],
                                         start=(ki == 0), stop=(ki == len(kt_sizes) - 1))
                    ob = pd.tile([128, F1], BF, tag="xl1o")
                    if t % 2 == 0:
                        nc.vector.tensor_copy(out=ob[:], in_=ps[:])
                    else:
                        nc.scalar.copy(out=ob[:], in_=ps[:])
                    nc.sync.dma_start(out=xl1_full.ap()[t * 128:(t + 1) * 128, :],
                                      in_=ob[:])

            # ---------- helper: edge phase ----------
            def edge_phase(F, xfull, esrc_sb, ah_sb, eh_mats, meanm, shift_ap,
                           bias_col, out_cb, layer):
                nhalf = F // 128
                groups = {}
                for s, (b, nstart) in enumerate(subtiles):
                    groups.setdefault(nstart // 128, []).append((s, b, nstart % 128))
                with tc.tile_pool(name=f"ge{layer}", bufs=6) as gp, \
                     tc.tile_pool(name=f"gz{layer}", bufs=2, space="PSUM") as gz, \
                     tc.tile_pool(name=f"gl{layer}", bufs=1, space="PSUM") as gl, \
                     tc.tile_pool(name=f"gn{layer}", bufs=2, space="PSUM") as gn, \
                     tc.tile_pool(name=f"gd{layer}", bufs=1, space="PSUM") as gd, \
                     tc.tile_pool(name=f"gs{layer}", bufs=4) as gs:
                    for g in range(NG):
                        subs = groups[g]
                        numT = gn.tile([128, nhalf * 128], F32, space="PSUM",
                                       tag="numT")
                        denT = gd.tile([8, 128], F32, space="PSUM", tag="denT")
                        # batches of 4 subtiles
                        for b0 in range(0, len(subs), 4):
                            batch = subs[b0:b0 + 4]
                            nb = len(batch)
                            zts = []
                            for h in range(nhalf):
                                zth = gz.tile([128, nb * 128], F32, space="PSUM",
                                              tag=f"zt{h}", name=f"zt{h}")
                                zts.append(zth)
                            lg = gl.tile([128, nb * 8], F32, space="PSUM", tag="lg")
                            xls_b = []
                            for si, (s, b, noff) in enumerate(batch):
                                xls = gp.tile([128, F], BF, tag="xls")
                                nc.gpsimd.indirect_dma_start(
                                    out=xls[:], out_offset=None,
                                    in_=xfull.ap(),
                                    in_offset=bass.IndirectOffsetOnAxis(
                                        ap=esrc_sb[:, s:s + 1], axis=0))
                                xls_b.append(xls)
                                nps = 128 // b
                                for h in range(nhalf):
                                    zsl = zts[h][:, si * 128:(si + 1) * 128]
                                    nc.tensor.matmul(
                                        out=zsl, lhsT=xls[:, h * 128:(h + 1) * 128],
                                        rhs=ident[:],
                                        start=True, stop=False)
                                    xr_ap = (xr1T[g][h] if layer == 1 else xr2T[g])
                                    rep = xr_ap[:, noff:noff + nps, None] \
                                        .broadcast_to([128, nps, b])
                                    nc.tensor.matmul(out=zsl, lhsT=ident[:],
                                                     rhs=rep, start=False,
                                                     stop=True)
                            es = []
                            for h in range(nhalf):
                                # lrelu(z) = z + relu(-0.8 z)
                                r8 = gs.tile([128, nb * 128], BF, tag=f"r8{h}",
                                             name=f"r8{h}")
                                nc.scalar.activation(
                                    out=r8[:], in_=zts[h][:],
                                    func=mybir.ActivationFunctionType.Relu,
                                    scale=nslope[:, :1])
                                e_sb = gs.tile([128, nb * 128], BF, tag=f"es{h}",
                                               name=f"es{h}")
                                nc.vector.tensor_tensor(
                                    out=e_sb[:], in0=zts[h][:], in1=r8[:],
                                    op=mybir.AluOpType.add)
                                es.append(e_sb)
                            for si, (s, b, noff) in enumerate(batch):
                                for h in range(nhalf):
                                    nc.tensor.matmul(
                                        out=lg[:, si * 8:(si + 1) * 8],
                                        lhsT=es[h][:, si * 128:(si + 1) * 128],
                                        rhs=ah_sb[h][:],
                                        start=(h == 0), stop=(h == nhalf - 1))
                            w4 = gs.tile([128, nb * 8], BF, tag="w4")
                            nc.scalar.activation(
                                out=w4[:], in_=lg[:],
                                func=mybir.ActivationFunctionType.Exp,
                                bias=shift_ap[:, :1])
                            for si, (s, b, noff) in enumerate(batch):
                                nps = 128 // b
                                S_t = gs.tile([128, nps], BF, tag="S")
                                nc.vector.tensor_scalar(
                                    out=S_t[:], in0=io_b[b][:],
                                    scalar1=edsts_sb[:, s:s + 1], scalar2=None,
                                    op0=mybir.AluOpType.is_equal)
                                y = gp.tile([128, F], BF, tag="y")
                                wv = w4[:, si * 8:(si + 1) * 8, None] \
                                    .broadcast_to([128, 8, F // 8])
                                nc.vector.tensor_tensor(
                                    out=y[:].rearrange("p (a b) -> p a b", a=8),
                                    in0=xls_b[si][:].rearrange(
                                        "p (a b) -> p a b", a=8),
                                    in1=wv, op=mybir.AluOpType.mult)
                                for h in range(nhalf):
                                    nc.tensor.matmul(
                                        out=numT[:, h * 128 + noff:h * 128 + noff + nps],
                                        lhsT=y[:, h * 128:(h + 1) * 128],
                                        rhs=S_t[:], start=True, stop=True)
                                nc.tensor.matmul(
                                    out=denT[0:8, noff:noff + nps],
                                    lhsT=w4[:, si * 8:(si + 1) * 8],
                                    rhs=S_t[:], start=True, stop=True)
                        # ---- group epilogue ----
                        drec = gs.tile([8, 128], F32, tag="drec")
                        nc.vector.reciprocal(drec[:], denT[:])
                        drecb = gs.tile([8, 128], BF, tag="drecb")
                        nc.vector.tensor_copy(out=drecb[:], in_=drec[:])
                        onts = []
                        for h in range(nhalf):
                            rexp = gz.tile([128, 128], F32, space="PSUM",
                                           tag=f"zt{h}")
                            nc.tensor.matmul(out=rexp[:], lhsT=eh_mats[h],
                                             rhs=drecb[:], start=True, stop=True)
                            rexpb = gs.tile([128, 128], BF, tag=f"rexpb{h}",
                                            name=f"rexpb{h}")
                            nc.scalar.copy(out=rexpb[:], in_=rexp[:])
                            ont = gs.tile([128, 128], BF, tag=f"ont{h}",
                                          name=f"ont{h}")
                            nc.vector.tensor_tensor(
                                out=ont[:], in0=numT[:, h * 128:(h + 1) * 128],
                                in1=rexpb[:], op=mybir.AluOpType.mult)
                            onts.append(ont)
                        cdim = C1 if layer == 1 else C2
                        ot = gl.tile([cdim, 128], F32, space="PSUM", tag="lg")
                        for h in range(nhalf):
                            nc.tensor.matmul(out=ot[:], lhsT=meanm[:, :cdim],
                                             rhs=onts[h][:], start=(h == 0),
                                             stop=(h == nhalf - 1))
                        out_cb(g, ot, bias_col)

            # ---------- phase B: layer-1 edges -> h1T ----------
            h1T = rpool.tile([HID, NODES_PAD], BF)
            oB = ctx.enter_context(tc.tile_pool(name="oB", bufs=2))

            def l1_out(g, ot_psum, bias_col):
                hrel = oB.tile([HID, 128], BF, tag="hrel")
                nc.scalar.activation(out=hrel[:], in_=ot_psum[:],
                                     func=mybir.ActivationFunctionType.Relu,
                                     bias=bias_col[:, :1])
                nc.vector.tensor_tensor(out=h1T[:, g * 128:(g + 1) * 128],
                                        in0=hrel[:],
                                        in1=mask32_sb[:, g * 128:(g + 1) * 128],
                                        op=mybir.AluOpType.mult)

            edge_phase(F1, xl1_full, esrc1_sb, a1_sb, e1h, mean1, msh1,
                       bias1t, l1_out, layer=1)

            # ---------- phase C: AllGather h1T + BN2 + xl2 + xr2T ----------
            with tc.tile_pool(name="pE", bufs=2) as pe:
                st2 = pe.tile([HID, 2], F32, tag="st2")
                nc.vector.tensor_reduce(out=st2[:, 0:1], in_=h1T[:],
                                        axis=mybir.AxisListType.X,
                                        op=mybir.AluOpType.add)
                scr2 = pe.tile([HID, NODES_PAD], BF, tag="scr2")
                nc.scalar.activation(out=scr2[:], in_=h1T[:],
                                     func=mybir.ActivationFunctionType.Square,
                                     accum_out=st2[:, 1:2])
                nc.sync.dma_start(out=ag_in.ap()[0:HID, :], in_=h1T[:])
                # stats rows (f32 bitcast into bf16 row space)
                nc.sync.dma_start(out=ag_in.ap()[HID:HID + 1, 0:2 * HID],
                                  in_=st2[:, 0:1].bitcast(BF))
                nc.sync.dma_start(out=ag_in.ap()[HID + 1:HID + 2, 0:2 * HID],
                                  in_=st2[:, 1:2].bitcast(BF))
            nc.gpsimd.collective_compute(
                "AllGather", mybir.AluOpType.bypass,
                ins=[ag_in.ap()], outs=[ag_out.ap()],
                replica_groups=[list(range(NCORES))])

            with tc.tile_pool(name="pF", bufs=1) as pf:
                # gather stat rows: each row HID f32 (=2*HID bf16)
                s2sum = pf.tile([HID, NCORES], F32, tag="s2sum")
                s2sq = pf.tile([HID, NCORES], F32, tag="s2sq")
                agf = ag_out.ap().bitcast(F32)  # [272, NODES_PAD//2]
                for c in range(NCORES):
                    r = c * (HID + 2) + HID
                    nc.sync.dma_start(out=s2sum[:, c:c + 1],
                                      in_=agf[r:r + 1, 0:HID])
                    nc.sync.dma_start(out=s2sq[:, c:c + 1],
                                      in_=agf[r + 1:r + 2, 0:HID])
                stg = pf.tile([HID, 2], F32, tag="stg2")
                nc.vector.tensor_reduce(out=stg[:, 0:1], in_=s2sum[:],
                                        axis=mybir.AxisListType.X,
                                        op=mybir.AluOpType.add)
                nc.vector.tensor_reduce(out=stg[:, 1:2], in_=s2sq[:],
                                        axis=mybir.AxisListType.X,
                                        op=mybir.AluOpType.add)
                gb = pf.tile([HID, 2], F32, tag="gb2")
                nc.sync.dma_start(out=gb[:], in_=din["gb2"].ap())
                mean = pf.tile([HID, 1], F32, tag="mean2")
                nc.vector.tensor_scalar(out=mean[:], in0=stg[:, 0:1],
                                        scalar1=RECIP_N, scalar2=None,
                                        op0=mybir.AluOpType.mult)
                q = pf.tile([HID, 1], F32, tag="q2")
                nc.vector.tensor_scalar(out=q[:], in0=stg[:, 1:2],
                                        scalar1=RECIP_N, scalar2=None,
                                        op0=mybir.AluOpType.mult)
                m2 = pf.tile([HID, 1], F32, tag="m22")
                nc.vector.tensor_tensor(out=m2[:], in0=mean[:], in1=mean[:],
                                        op=mybir.AluOpType.mult)
                var = pf.tile([HID, 1], F32, tag="var2")
                nc.vector.tensor_tensor(out=var[:], in0=q[:], in1=m2[:],
                                        op=mybir.AluOpType.subtract)
                sd = pf.tile([HID, 1], F32, tag="sd2")
                nc.scalar.activation(out=sd[:], in_=var[:],
                                     func=mybir.ActivationFunctionType.Sqrt,
                                     bias=epsb[:HID, :1])
                rstd = pf.tile([HID, 1], F32, tag="rstd2")
                nc.vector.reciprocal(rstd[:], sd[:])
                s2 = pf.tile([HID, 1], F32, tag="s2")
                nc.vector.tensor_tensor(out=s2[:], in0=gb[:, 0:1], in1=rstd[:],
                                        op=mybir.AluOpType.mult)
                ms = pf.tile([HID, 1], F32, tag="ms2")
                nc.vector.tensor_tensor(out=ms[:], in0=mean[:], in1=s2[:],
                                        op=mybir.AluOpType.mult)
                t2 = pf.tile([HID, 1], F32, tag="t2")
                nc.vector.tensor_tensor(out=t2[:], in0=gb[:, 1:2], in1=ms[:],
                                        op=mybir.AluOpType.subtract)
                # scaled weights
                wl2f = pf.tile([HID, F2], F32, tag="wl2f")
                nc.sync.dma_start(out=wl2f[:], in_=din["Wl2"].ap())
                wr2f = pf.tile([HID, F2], F32, tag="wr2f")
                nc.sync.dma_start(out=wr2f[:], in_=din["Wr2"].ap())
                wl2s = rpool.tile([HID, F2], BF)
                nc.vector.tensor_scalar(out=wl2s[:], in0=wl2f[:],
                                        scalar1=s2[:, :1], scalar2=None,
                                        op0=mybir.AluOpType.mult)
                wr2s = rpool.tile([HID, F2], BF)
                nc.vector.tensor_scalar(out=wr2s[:], in0=wr2f[:],
                                        scalar1=s2[:, :1], scalar2=None,
                                        op0=mybir.AluOpType.mult)
                t2b = pf.tile([HID, 1], BF, tag="t2b")
                nc.vector.tensor_copy(out=t2b[:], in_=t2[:])
                wsum = pf.tile([HID, F2], BF, tag="wsum")
                nc.vector.tensor_tensor(out=wsum[:], in0=wl2f[:], in1=wr2f[:],
                                        op=mybir.AluOpType.add)
                with tc.tile_pool(name="pFp", bufs=1, space="PSUM") as pfp:
                    bps = pfp.tile([1, F2], F32, space="PSUM")
                    nc.tensor.matmul(out=bps[:], lhsT=t2b[:], rhs=wsum[:],
                                     start=True, stop=True)
                    bias22 = rpool.tile([1, F2], BF)
                    nc.vector.tensor_copy(out=bias22[:], in_=bps[:])
                with tc.tile_pool(name="pFq", bufs=1, space="PSUM") as pfq:
                    wmf2 = pf.tile([HID, OUT], F32, tag="wmf2")
                    nc.sync.dma_start(out=wmf2[:], in_=din["Wl2m"].ap())
                    wmb2 = pf.tile([HID, OUT], BF, tag="wmb2")
                    nc.vector.tensor_copy(out=wmb2[:], in_=wmf2[:])
                    blp2 = pfq.tile([1, OUT], F32, space="PSUM")
                    nc.tensor.matmul(out=blp2[:], lhsT=t2b[:], rhs=wmb2[:],
                                     start=True, stop=True)
                    blr2 = pf.tile([1, OUT], F32, tag="blr2")
                    nc.vector.tensor_copy(out=blr2[:], in_=blp2[:])
                    nc.sync.dma_start(out=bl2_d.ap(), in_=blr2[:])
                blc2 = rpool.tile([OUT, 1], F32)
                nc.sync.dma_start(out=blc2[:], in_=bl2_d.ap())
                bias2t = rpool.tile([OUT, 1], F32)
                nc.vector.tensor_tensor(out=bias2t[:], in0=blc2[:], in1=b2c_sb[:],
                                        op=mybir.AluOpType.add)

            # xl2_full
            with tc.tile_pool(name="pG", bufs=3) as pg, \
                 tc.tile_pool(name="pGp", bufs=2, space="PSUM") as pgp:
                for t in range(n_xl2_tiles):
                    c_src = (t * 128) // NODES_PAD
                    off = (t * 128) % NODES_PAD
                    lhs2 = pg.tile([HID, 128], BF, tag="xl2l")
                    nc.sync.dma_start(
                        out=lhs2[:],
                        in_=ag_out.ap()[c_src * (HID + 2):c_src * (HID + 2) + HID,
                                        off:off + 128])
                    ps = pgp.tile([128, F2], F32, space="PSUM", tag="xl2p")
                    nc.tensor.matmul(out=ps[:], lhsT=lhs2[:],
                                     rhs=wl2s[:], start=True, stop=True)
                    ob = pg.tile([128, F2], BF, tag="xl2o")
                    if t % 2 == 0:
                        nc.vector.tensor_copy(out=ob[:], in_=ps[:])
                    else:
                        nc.scalar.copy(out=ob[:], in_=ps[:])
                    nc.sync.dma_start(out=xl2_full.ap()[t * 128:(t + 1) * 128, :],
                                      in_=ob[:])
            # xr2T resident
            xr2T = []
            with tc.tile_pool(name="pH", bufs=2, space="PSUM") as ph:
                for g in range(NG):
                    ps = ph.tile([128, 128], F32, space="PSUM", tag="xr2p")
                    nc.tensor.matmul(out=ps[:], lhsT=wr2s[:],
                                     rhs=h1T[:, g * 128:(g + 1) * 128],
                                     start=True, stop=False)
                    nc.tensor.matmul(out=ps[:], lhsT=bias22[:], rhs=ones_row[:],
                                     start=False, stop=True)
                    t = rpool.tile([128, 128], BF, tag=f"xr2T_{g}")
                    if g % 2 == 0:
                        nc.vector.tensor_copy(out=t[:], in_=ps[:])
                    else:
                        nc.scalar.copy(out=t[:], in_=ps[:])
                    xr2T.append(t)

            # ---------- phase D: layer-2 edges -> outT ----------
            oD = ctx.enter_context(tc.tile_pool(name="oD", bufs=2))

            def l2_out(g, ot_psum, bias_col):
                ob = oD.tile([OUT, 128], F32, tag="ob")
                nc.scalar.activation(out=ob[:], in_=ot_psum[:],
                                     func=mybir.ActivationFunctionType.Identity,
                                     bias=bias_col[:, :1])
                nc.sync.dma_start(out=outT.ap()[:, g * 128:(g + 1) * 128],
                                  in_=ob[:])

            edge_phase(F2, xl2_full, esrc2_sb, a2_sb, e2h, mean2, msh2,
                       bias2t, l2_out, layer=2)

    nc.compile()
    return nc


_CACHE = {}


def _get_nc(cfg, meta):
    key = (cfg.N, cfg.IN, cfg.HID, cfg.OUT, meta["NSUB"], meta["NODES_PAD"])
    if key not in _CACHE:
        _CACHE[key] = _build(cfg, meta)
    return _CACHE[key]


def run(cfg, inputs):
    x = np.asarray(inputs["x"], np.float32)
    ei = np.asarray(inputs["edge_index"], np.int32)
    W = {k: np.asarray(inputs[k], np.float32) for k in
         ("Wl1", "Wr1", "att1", "b1", "gamma1", "beta1",
          "Wl2", "Wr2", "att2", "b2", "gamma2", "beta2")}
    meta = _preprocess(cfg, x, ei, W)
    nc = _get_nc(cfg, meta)
    res = run_bass_kernel_spmd(nc, meta["in_maps"], core_ids=list(range(NCORES)))
    out = np.empty((cfg.N, cfg.OUT), np.float32)
    proc = meta["proc"]
    for c in range(NCORES):
        oT = res.results[c]["outT"]      # [OUT, NODES_PAD]
        sel = proc[c] >= 0
        out[c * cfg.NL + proc[c][sel]] = oT[:, sel].T
    return out, meta, nc


def kernel(**inputs):
    cfg = Cfg(50000, 200, 32, 16, m1=8.0, m2=10.0)
    out, _, _ = run(cfg, inputs)
    return out



# revision 8
# speedup vs baseline: 6.9036x; 6.9036x over previous
"""Distributed GATv2 (2 layers + BN) Bass kernel for 8 trn2 NeuronCores.

Strategy: nodes partitioned by range across 8 cores (dst-ownership).
Each core:
  - computes BN1 stats partials -> AllReduce -> folds BN into Wl1/Wr1
  - computes xl1 = bn(x)@Wl1s for ALL nodes (bf16 table in local DRAM,
    mirror layout: row v stored at v^32768 so int16 dma_gather indices
    cover the full node range when gathering from base+32768)
  - computes xr1T (feat-major, + folded biases) for its own nodes
  - edge phase L1: batched dma_gather (1024 edges / 8 subtiles per op),
    4-subtile same-bucket compute batches: batched xr broadcast-add on
    PE, per-subtile feat-major transpose-accumulate, LeakyReLU on
    ACT+DVE, logits via PE against block-diag att, exp with a global
    shift (softmax-invariant), transposed segment-sums numT/denT via
    one-hot matmuls, feat-major epilogue -> h1T (bf16)
  - one AllGather of h1T (+BN2 stat partials packed in 2 extra rows)
  - BN2 fold, xl2 mirror table for all nodes, edge phase L2 (same)
Output per core: outT [16, NODES_PAD] f32; host unpermutes/concats.
"""
import sys
import numpy as np

sys.path.insert(0, "/opt/trn_rl_repo")

import concourse.bass as bass          # noqa: E402
import concourse.bacc as bacc          # noqa: E402
import concourse.tile as tile          # noqa: E402
from concourse import mybir            # noqa: E402
from concourse.bass_utils import run_bass_kernel_spmd  # noqa: E402
from concourse.masks import make_identity  # noqa: E402

F32 = mybir.dt.float32
BF = mybir.dt.bfloat16
I32 = mybir.dt.int32
I16 = mybir.dt.int16
NPBF = mybir.dt.np(BF)

NCORES = 8
HEADS = 8
BN_EPS = 1e-5
NEG_SLOPE = 0.2
BUCKETS = (4, 8, 16, 32, 64, 128)
MIR = 32768          # mirror offset for int16 dma_gather indices
TABR = 2 * MIR       # mirror table rows
GSUB = 8             # subtiles per dma_gather (1024 edges)
CSUB = 4             # subtiles per compute batch


class Cfg:
    def __init__(self, n_nodes, in_dim, hid, out, m1, m2):
        self.N = n_nodes
        self.IN = in_dim
        self.HID = hid
        self.OUT = out
        self.F1 = HEADS * hid
        self.F2 = HEADS * out
        self.M1 = m1          # logit shift (softmax-invariant), layer 1
        self.M2 = m2
        self.NL = n_nodes // NCORES
        self.NT_PAD = ((n_nodes + 127) // 128) * 128
        self.KT = [min(128, in_dim), max(0, in_dim - 128)]  # K tiles for IN


def _schedule(cfg, deg_per_core):
    """Bucketed subtile schedule, uniform across cores.

    Each bucket's subtile count is a multiple of CSUB so compute batches
    are same-bucket and group-aligned; total NSUB is a multiple of GSUB
    and NODES_PAD a multiple of 128.
    """
    t_b = {}
    for b in BUCKETS:
        lo = 0 if b == BUCKETS[0] else BUCKETS[BUCKETS.index(b) - 1]
        cnt = max(int(((d > lo) & (d <= b)).sum()) for d in deg_per_core)
        nps = 128 // b
        unit = CSUB * nps
        t_b[b] = ((cnt + unit - 1) // unit) * unit
    # pad with bucket-128 / bucket-64 batches until NODES_PAD % 128 == 0
    # and NSUB % GSUB == 0
    def tots():
        nodes = sum(t_b.values())
        subs = sum(t_b[b] // (128 // b) for b in BUCKETS)
        return nodes, subs
    solved = False
    for c in range(4):
        for a in range(64):
            nodes, subs = tots()
            nodes += 4 * a + 8 * c
            subs += 4 * a + 4 * c
            if nodes % 128 == 0 and subs % GSUB == 0:
                t_b[128] += 4 * a
                t_b[64] += 8 * c
                solved = True
                break
        if solved:
            break
    assert solved
    nodes_pad, nsub = tots()
    assert nodes_pad % 128 == 0 and nsub % GSUB == 0
    subtiles = []   # (bucket, node_start)
    pos = 0
    for b in BUCKETS:
        nps = 128 // b
        for k in range(t_b[b] // nps):
            subtiles.append((b, pos))
            pos += nps
    assert pos == nodes_pad
    return t_b, nodes_pad, subtiles


def _pack_idx16(src_u16):
    """[NSUB, 128] uint16 -> [128, (NSUB//GSUB)*64] int16 dma_gather layout.

    Flat edge i of a gather batch lands at out[i%128, i//128]; its index
    is read from idxs[i%16, i//16] (16-partition wrap, replicated x8)."""
    nsub = src_u16.shape[0]
    ng = nsub // GSUB
    u = src_u16.reshape(ng, GSUB, 128)
    idx16 = np.zeros((ng, 16, GSUB * 8), np.uint16)
    P = np.arange(128)
    for si in range(GSUB):
        idx16[:, P % 16, si * 8 + P // 16] = u[:, si, :]
    full = np.tile(idx16, (1, 8, 1))          # [ng, 128, 64]
    out = full.transpose(1, 0, 2).reshape(128, ng * GSUB * 8)
    return out.view(np.int16)


def _preprocess(cfg, x, edge_index, W):
    N, NL = cfg.N, cfg.NL
    src = np.concatenate([edge_index[0], np.arange(N, dtype=np.int32)])
    dst = np.concatenate([edge_index[1], np.arange(N, dtype=np.int32)])
    order = np.argsort(dst, kind="stable")
    src, dst = src[order], dst[order]
    deg = np.bincount(dst, minlength=N)
    starts = np.zeros(N + 1, np.int64)
    np.cumsum(deg, out=starts[1:])
    deg_pc = [deg[c * NL:(c + 1) * NL] for c in range(NCORES)]
    t_b, NODES_PAD, subtiles = _schedule(cfg, deg_pc)
    NSUB = len(subtiles)
    NG = NODES_PAD // 128

    # per-core node processing order (bucket-sorted) and dummy slots (-1)
    proc = np.full((NCORES, NODES_PAD), -1, np.int64)   # proc pos -> local node
    ppos = np.full((NCORES, NL), -1, np.int64)          # local node -> proc pos
    for c in range(NCORES):
        pos = 0
        d = deg_pc[c]
        for b in BUCKETS:
            lo = 0 if b == BUCKETS[0] else BUCKETS[BUCKETS.index(b) - 1]
            ids = np.nonzero((d > lo) & (d <= b))[0]
            proc[c, pos:pos + len(ids)] = ids
            ppos[c, ids] = pos + np.arange(len(ids))
            pos += t_b[b]
    # storage index of a global node: core*NODES_PAD + ppos
    store = np.empty(N, np.int64)
    for c in range(NCORES):
        store[c * NL:(c + 1) * NL] = c * NODES_PAD + ppos[c]

    esrc1 = np.zeros((NCORES, NSUB, 128), np.uint16)
    esrc2 = np.zeros((NCORES, NSUB, 128), np.uint16)
    edsts = np.full((NCORES, 128, NSUB), -1.0, np.float32)
    for c in range(NCORES):
        for s, (b, nstart) in enumerate(subtiles):
            nps = 128 // b
            for slot in range(nps):
                v = proc[c, nstart + slot]
                if v < 0:
                    # dummy keep-alive edge so den > 0
                    edsts[c, slot * b, s] = slot
                    continue
                gv = c * NL + int(v)
                e0 = starts[gv]
                dv = int(deg[gv])
                p0 = slot * b
                esrc1[c, s, p0:p0 + dv] = src[e0:e0 + dv].astype(np.uint16)
                esrc2[c, s, p0:p0 + dv] = store[src[e0:e0 + dv]].astype(
                    np.uint16)
                edsts[c, p0:p0 + dv, s] = slot
        # dma_gather drops a TRAILING run of sign-negative (>=32768)
        # int16 indices: make position 127 of the last subtile of every
        # gather batch end with a safe (<32768) index in both layers'
        # encodings. Edge order within a slot is free (softmax-invariant)
        # and may differ between the two tables.
        for gb in range(NSUB // GSUB):
            s = gb * GSUB + GSUB - 1
            b, nstart = subtiles[s]
            nps = 128 // b
            p0 = (nps - 1) * b

            def slot_ok(sl):
                q0 = sl * b
                if edsts[c, q0 + b - 1, s] < 0:     # not full -> dummy end
                    return True
                return ((esrc1[c, s, q0:q0 + b] < MIR).any() and
                        (esrc2[c, s, q0:q0 + b] < MIR).any())

            if not slot_ok(nps - 1):
                for sl in range(nps - 1):
                    if slot_ok(sl):
                        q0 = sl * b
                        for tab in (esrc1[c, s], esrc2[c, s]):
                            tmp = tab[q0:q0 + b].copy()
                            tab[q0:q0 + b] = tab[p0:p0 + b]
                            tab[p0:p0 + b] = tmp
                        dcol = edsts[c, :, s]
                        da = (dcol[q0:q0 + b] >= 0).copy()
                        db_ = (dcol[p0:p0 + b] >= 0).copy()
                        dcol[q0:q0 + b] = np.where(db_, float(sl), -1.0)
                        dcol[p0:p0 + b] = np.where(da, float(nps - 1), -1.0)
                        # keep-alive if a swapped-in slot is empty
                        if not db_.any():
                            dcol[q0] = float(sl)
                        if not da.any():
                            dcol[p0] = float(nps - 1)
                        n1 = proc[c, nstart + sl]
                        proc[c, nstart + sl] = proc[c, nstart + nps - 1]
                        proc[c, nstart + nps - 1] = n1
                        for nn, pp in ((proc[c, nstart + sl], nstart + sl),
                                       (proc[c, nstart + nps - 1],
                                        nstart + nps - 1)):
                            if nn >= 0:
                                ppos[c, nn] = pp
                        break
                else:
                    raise AssertionError("no safe tail slot")
            for tab in (esrc1[c, s], esrc2[c, s]):
                if tab[127] >= MIR:
                    seg = tab[p0:128]
                    safe = np.nonzero(seg < MIR)[0]
                    j = p0 + safe[-1]
                    tab[127], tab[j] = tab[j], tab[127]

    # inputs per core
    xT = np.zeros((cfg.IN, cfg.NT_PAD), NPBF)
    xT[:, :N] = x.T.astype(NPBF)
    in_maps = []
    A1 = np.zeros((cfg.F1, HEADS), np.float32)
    for h in range(HEADS):
        A1[h * cfg.HID:(h + 1) * cfg.HID, h] = W["att1"][h]
    A2 = np.zeros((cfg.F2, HEADS), np.float32)
    for h in range(HEADS):
        A2[h * cfg.OUT:(h + 1) * cfg.OUT, h] = W["att2"][h]
    # iota tables per bucket, packed [128, sum(nps)] f32
    iob = np.zeros((128, sum(128 // b for b in BUCKETS)), np.float32)
    off = 0
    iob_off = {}
    for b in BUCKETS:
        nps = 128 // b
        iob[:, off:off + nps] = np.arange(nps)[None, :]
        iob_off[b] = off
        off += nps
    for c in range(NCORES):
        xTo = np.zeros((cfg.IN, NODES_PAD), NPBF)
        sel = proc[c] >= 0
        xTo[:, sel] = x[c * NL + proc[c][sel]].T.astype(NPBF)
        mask32 = np.zeros((cfg.HID, NODES_PAD), NPBF)
        mask32[:, sel] = 1.0
        mean1m = np.zeros((128, cfg.HID), np.float32)
        mean1m[np.arange(128), np.arange(128) % cfg.HID] = 0.125
        mean2m = np.zeros((128, cfg.OUT), np.float32)
        mean2m[np.arange(128), np.arange(128) % cfg.OUT] = 0.125
        e1m = np.zeros((8, cfg.F1), np.float32)
        e1m[np.arange(cfg.F1) // cfg.HID, np.arange(cfg.F1)] = 1.0
        e2m = np.zeros((8, cfg.F2), np.float32)
        e2m[np.arange(cfg.F2) // cfg.OUT, np.arange(cfg.F2)] = 1.0
        in_maps.append({
            "xT": xT, "xTo": xTo, "mask32": mask32,
            "mean1m": mean1m, "mean2m": mean2m, "e1m": e1m, "e2m": e2m,
            "iob": iob,
            "esrc1g": np.ascontiguousarray(_pack_idx16(esrc1[c])),
            "esrc2g": np.ascontiguousarray(_pack_idx16(esrc2[c])),
            "edsts": np.ascontiguousarray(edsts[c]),
            "Wl1m": W["Wl1"].reshape(cfg.IN, HEADS, cfg.HID).mean(1).astype(np.float32),
            "Wl2m": W["Wl2"].reshape(cfg.HID, HEADS, cfg.OUT).mean(1).astype(np.float32),
            "Wl1": W["Wl1"].astype(np.float32),
            "Wr1": W["Wr1"].astype(np.float32),
            "Wl2": W["Wl2"].astype(np.float32),
            "Wr2": W["Wr2"].astype(np.float32),
            "A1": A1, "A2": A2,
            "gb1": np.stack([W["gamma1"], W["beta1"]], 1).astype(np.float32),
            "gb2": np.stack([W["gamma2"], W["beta2"]], 1).astype(np.float32),
            "b1c": W["b1"].reshape(-1, 1).astype(np.float32),
            "b2c": W["b2"].reshape(-1, 1).astype(np.float32),
        })
    meta = dict(NODES_PAD=NODES_PAD, NSUB=NSUB, NG=NG, subtiles=subtiles,
                proc=proc, in_maps=in_maps, iob_off=iob_off)
    return meta


def _build(cfg, meta):
    NODES_PAD, NSUB, NG = meta["NODES_PAD"], meta["NSUB"], meta["NG"]
    subtiles = meta["subtiles"]
    iob_off = meta["iob_off"]
    IN, F1, F2, HID, OUT = cfg.IN, cfg.F1, cfg.F2, cfg.HID, cfg.OUT
    K0, K1 = cfg.KT
    NTP = cfg.NT_PAD
    C1, C2 = HID, OUT
    NIDXC = (NSUB // GSUB) * GSUB * 8        # esrc16 columns
    n_xl1_tiles = NTP // 128
    n_xl2_rows = NCORES * NODES_PAD
    RECIP_N = 1.0 / cfg.N
    IOBW = sum(128 // b for b in BUCKETS)

    nc = bacc.Bacc("TRN2", target_bir_lowering=False, debug=False,
                   num_devices=NCORES)
    din = {}
    for name, shape, dt in [
            ("xT", [IN, NTP], BF), ("xTo", [IN, NODES_PAD], BF),
            ("mask32", [HID, NODES_PAD], BF),
            ("esrc1g", [128, NIDXC], I16), ("esrc2g", [128, NIDXC], I16),
            ("edsts", [128, NSUB], F32), ("iob", [128, IOBW], F32),
            ("Wl1", [IN, F1], F32), ("Wr1", [IN, F1], F32),
            ("Wl2", [HID, F2], F32), ("Wr2", [HID, F2], F32),
            ("A1", [F1, HEADS], F32), ("A2", [F2, HEADS], F32),
            ("gb1", [IN, 2], F32), ("gb2", [HID, 2], F32),
            ("b1c", [HID, 1], F32), ("b2c", [OUT, 1], F32),
            ("mean1m", [128, HID], F32), ("mean2m", [128, OUT], F32),
            ("Wl1m", [IN, HID], F32), ("Wl2m", [HID, OUT], F32),
            ("e1m", [8, F1], F32), ("e2m", [8, F2], F32)]:
        din[name] = nc.dram_tensor(name, shape, dt, kind="ExternalInput")
    outT = nc.dram_tensor("outT", [OUT, NODES_PAD], F32, kind="ExternalOutput")

    xl1_full = nc.dram_tensor("xl1_full", [TABR, F1], BF)
    xl2_full = nc.dram_tensor("xl2_full", [TABR, F2], BF)
    st1_in = nc.dram_tensor("st1_in", [IN, 2], F32)
    bl1_d = nc.dram_tensor("bl1_d", [1, HID], F32)
    bl2_d = nc.dram_tensor("bl2_d", [1, OUT], F32)
    st1_out = nc.dram_tensor("st1_out", [IN, 2], F32)
    ag_in = nc.dram_tensor("ag_in", [HID + 2, NODES_PAD], BF)
    ag_out = nc.dram_tensor("ag_out", [NCORES * (HID + 2), NODES_PAD], BF,
                            addr_space="Shared")

    import contextlib
    with tile.TileContext(nc) as tc:
        ctx = contextlib.ExitStack()
        with ctx:
            cpool = ctx.enter_context(tc.tile_pool(name="const", bufs=1))
            rpool = ctx.enter_context(tc.tile_pool(name="resident", bufs=1))

            # ---------- constants ----------
            ident = cpool.tile([128, 128], BF)
            make_identity(nc, ident[:])
            ones_row = cpool.tile([1, 128], BF)
            nc.vector.memset(ones_row[:], 1.0)
            epsb = cpool.tile([128, 1], F32, tag="epsb")
            nc.vector.memset(epsb[:], BN_EPS)
            msh1 = cpool.tile([128, 1], F32, tag="msh1")
            nc.vector.memset(msh1[:], -float(cfg.M1))
            msh2 = cpool.tile([128, 1], F32, tag="msh2")
            nc.vector.memset(msh2[:], -float(cfg.M2))
            nslope = cpool.tile([128, 1], F32, tag="nslope")
            nc.vector.memset(nslope[:], -(1.0 - NEG_SLOPE))
            iob_sb = cpool.tile([128, IOBW], F32, tag="iob")
            nc.sync.dma_start(out=iob_sb[:], in_=din["iob"].ap())
            io_b = {b: iob_sb[:, iob_off[b]:iob_off[b] + 128 // b]
                    for b in BUCKETS}

            def const_bf(name, shape, tagn):
                tf = cpool.tile(shape, F32, tag=tagn + "f", name=tagn + "f")
                nc.sync.dma_start(out=tf[:], in_=din[name].ap())
                tb = cpool.tile(shape, BF, tag=tagn, name=tagn)
                nc.vector.tensor_copy(out=tb[:], in_=tf[:])
                return tb
            mean1 = const_bf("mean1m", [128, C1], "mean1")
            mean2 = const_bf("mean2m", [128, C2], "mean2")
            e1full = const_bf("e1m", [8, F1], "e1m")
            e2full = const_bf("e2m", [8, F2], "e2m")
            e1h = [e1full[:, h * 128:(h + 1) * 128] for h in range(F1 // 128)]
            e2h = [e2full[:, h * 128:(h + 1) * 128] for h in range(F2 // 128)]

            # index/dst tables resident
            esrc1_sb = rpool.tile([128, NIDXC], I16)
            nc.sync.dma_start(out=esrc1_sb[:], in_=din["esrc1g"].ap())
            esrc2_sb = rpool.tile([128, NIDXC], I16)
            nc.sync.dma_start(out=esrc2_sb[:], in_=din["esrc2g"].ap())
            edsts_sb = rpool.tile([128, NSUB], F32)
            nc.sync.dma_start(out=edsts_sb[:], in_=din["edsts"].ap())
            # A1/A2 bf16
            a1_sb = []
            for h in range(F1 // 128):
                t = rpool.tile([128, HEADS], BF, tag=f"a1_{h}")
                tf = rpool.tile([128, HEADS], F32, tag=f"a1f_{h}")
                nc.sync.dma_start(out=tf[:], in_=din["A1"].ap()[h * 128:(h + 1) * 128, :])
                nc.vector.tensor_copy(out=t[:], in_=tf[:])
                a1_sb.append(t)
            a2_sb = []
            for h in range(F2 // 128):
                t = rpool.tile([128, HEADS], BF, tag=f"a2_{h}")
                tf = rpool.tile([128, HEADS], F32, tag=f"a2f_{h}")
                nc.sync.dma_start(out=tf[:], in_=din["A2"].ap()[h * 128:(h + 1) * 128, :])
                nc.vector.tensor_copy(out=t[:], in_=tf[:])
                a2_sb.append(t)
            b1c_sb = rpool.tile([HID, 1], F32)
            nc.sync.dma_start(out=b1c_sb[:], in_=din["b1c"].ap())
            b2c_sb = rpool.tile([OUT, 1], F32)
            nc.sync.dma_start(out=b2c_sb[:], in_=din["b2c"].ap())
            mask32_sb = rpool.tile([HID, NODES_PAD], BF)
            nc.sync.dma_start(out=mask32_sb[:], in_=din["mask32"].ap())

            # xTo resident tiles (feat-major own nodes)
            kt_sizes = [K0] + ([K1] if K1 else [])
            xTo_sb = []
            for ki, ks in enumerate(kt_sizes):
                t = rpool.tile([ks, NODES_PAD], BF, tag=f"xTo{ki}")
                nc.sync.dma_start(out=t[:], in_=din["xTo"].ap()[ki * 128:ki * 128 + ks, :])
                xTo_sb.append(t)

            # ---------- phase A: BN1 stats + AllReduce ----------
            with tc.tile_pool(name="pA", bufs=2) as pa, \
                 tc.tile_pool(name="pAs", bufs=1) as pas:
                for ki, ks in enumerate(kt_sizes):
                    st = pa.tile([ks, 2], F32, tag="st")
                    nc.vector.tensor_reduce(out=st[:, 0:1], in_=xTo_sb[ki][:],
                                            axis=mybir.AxisListType.X,
                                            op=mybir.AluOpType.add)
                    scr = pas.tile([ks, NODES_PAD], BF, tag="scr")
                    nc.scalar.activation(out=scr[:], in_=xTo_sb[ki][:],
                                         func=mybir.ActivationFunctionType.Square,
                                         accum_out=st[:, 1:2])
                    nc.sync.dma_start(out=st1_in.ap()[ki * 128:ki * 128 + ks, :],
                                      in_=st[:])
            nc.gpsimd.collective_compute(
                "AllReduce", mybir.AluOpType.add,
                ins=[st1_in.ap()], outs=[st1_out.ap()],
                replica_groups=[list(range(NCORES))])

            # fold stats -> s1, t1 (per K tile) and scaled weights
            s1_t, t1_t = [], []
            wl1s, wr1s = [], []
            wl1f, wr1f = [], []
            with tc.tile_pool(name="pB", bufs=1) as pb:
                for ki, ks in enumerate(kt_sizes):
                    stg = pb.tile([ks, 2], F32, tag=f"stg{ki}")
                    nc.sync.dma_start(out=stg[:], in_=st1_out.ap()[ki * 128:ki * 128 + ks, :])
                    gb = pb.tile([ks, 2], F32, tag=f"gb{ki}")
                    nc.sync.dma_start(out=gb[:], in_=din["gb1"].ap()[ki * 128:ki * 128 + ks, :])
                    mean = pb.tile([ks, 1], F32, tag=f"mean{ki}")
                    nc.vector.tensor_scalar(out=mean[:], in0=stg[:, 0:1],
                                            scalar1=RECIP_N, scalar2=None,
                                            op0=mybir.AluOpType.mult)
                    q = pb.tile([ks, 1], F32, tag=f"q{ki}")
                    nc.vector.tensor_scalar(out=q[:], in0=stg[:, 1:2],
                                            scalar1=RECIP_N, scalar2=None,
                                            op0=mybir.AluOpType.mult)
                    m2 = pb.tile([ks, 1], F32, tag=f"m2{ki}")
                    nc.vector.tensor_tensor(out=m2[:], in0=mean[:], in1=mean[:],
                                            op=mybir.AluOpType.mult)
                    var = pb.tile([ks, 1], F32, tag=f"var{ki}")
                    nc.vector.tensor_tensor(out=var[:], in0=q[:], in1=m2[:],
                                            op=mybir.AluOpType.subtract)
                    sd = pb.tile([ks, 1], F32, tag=f"sd{ki}")
                    nc.scalar.activation(out=sd[:], in_=var[:],
                                         func=mybir.ActivationFunctionType.Sqrt,
                                         bias=epsb[:ks, :1])
                    rstd = pb.tile([ks, 1], F32, tag=f"rstd{ki}")
                    nc.vector.reciprocal(rstd[:], sd[:])
                    s1 = pb.tile([ks, 1], F32, tag=f"s1{ki}")
                    nc.vector.tensor_tensor(out=s1[:], in0=gb[:, 0:1], in1=rstd[:],
                                            op=mybir.AluOpType.mult)
                    ms = pb.tile([ks, 1], F32, tag=f"ms{ki}")
                    nc.vector.tensor_tensor(out=ms[:], in0=mean[:], in1=s1[:],
                                            op=mybir.AluOpType.mult)
                    t1 = pb.tile([ks, 1], F32, tag=f"t1{ki}")
                    nc.vector.tensor_tensor(out=t1[:], in0=gb[:, 1:2], in1=ms[:],
                                            op=mybir.AluOpType.subtract)
                    s1_t.append(s1)
                    t1_t.append(t1)
                    for wname, lst, lstf in (("Wl1", wl1s, wl1f), ("Wr1", wr1s, wr1f)):
                        wf = rpool.tile([ks, F1], F32, tag=f"{wname}f{ki}")
                        nc.sync.dma_start(out=wf[:], in_=din[wname].ap()[ki * 128:ki * 128 + ks, :])
                        ws = rpool.tile([ks, F1], BF, tag=f"{wname}s{ki}")
                        nc.vector.tensor_scalar(out=ws[:], in0=wf[:],
                                                scalar1=s1[:, :1], scalar2=None,
                                                op0=mybir.AluOpType.mult)
                        lst.append(ws)
                        lstf.append(wf)
                # bias12 = t1 @ (Wl1 + Wr1)  [1, F1]
                t1b = []
                for ki, ks in enumerate(kt_sizes):
                    tb = pb.tile([ks, 1], BF, tag=f"t1b{ki}")
                    nc.vector.tensor_copy(out=tb[:], in_=t1_t[ki][:])
                    t1b.append(tb)
                with tc.tile_pool(name="pBp", bufs=1, space="PSUM") as pbp:
                    bps = pbp.tile([1, F1], F32, space="PSUM")
                    first = True
                    for ki, ks in enumerate(kt_sizes):
                        for wf in (wl1f[ki], wr1f[ki]):
                            wb = pb.tile([ks, F1], BF, tag=f"wb{ki}")
                            nc.vector.tensor_copy(out=wb[:], in_=wf[:])
                            nc.tensor.matmul(out=bps[:], lhsT=t1b[ki][:], rhs=wb[:],
                                             start=first, stop=(ki == len(kt_sizes) - 1 and wf is wr1f[ki]))
                            first = False
                    bias12 = rpool.tile([1, F1], BF)
                    nc.vector.tensor_copy(out=bias12[:], in_=bps[:])
                with tc.tile_pool(name="pBq", bufs=1, space="PSUM") as pbq:
                    blp = pbq.tile([1, HID], F32, space="PSUM")
                    for ki, ks in enumerate(kt_sizes):
                        wmf = pb.tile([ks, HID], F32, tag=f"wmf{ki}",
                                      name=f"wmf{ki}")
                        nc.sync.dma_start(
                            out=wmf[:],
                            in_=din["Wl1m"].ap()[ki * 128:ki * 128 + ks, :])
                        wmb = pb.tile([ks, HID], BF, tag=f"wmb{ki}",
                                      name=f"wmb{ki}")
                        nc.vector.tensor_copy(out=wmb[:], in_=wmf[:])
                        nc.tensor.matmul(out=blp[:], lhsT=t1b[ki][:], rhs=wmb[:],
                                         start=(ki == 0),
                                         stop=(ki == len(kt_sizes) - 1))
                    blr = pb.tile([1, HID], F32, tag="blr")
                    nc.vector.tensor_copy(out=blr[:], in_=blp[:])
                    nc.sync.dma_start(out=bl1_d.ap(), in_=blr[:])
                blc = rpool.tile([HID, 1], F32)
                nc.sync.dma_start(out=blc[:], in_=bl1_d.ap())
                bias1t = rpool.tile([HID, 1], F32)
                nc.vector.tensor_tensor(out=bias1t[:], in0=blc[:], in1=b1c_sb[:],
                                        op=mybir.AluOpType.add)

            # ---------- phase A2: xr1T resident (feat-major, own nodes) ----------
            xr1T = []
            with tc.tile_pool(name="pC", bufs=2, space="PSUM") as pc, \
                 tc.tile_pool(name="pCs", bufs=2) as pcs:
                for g in range(NG):
                    halves = []
                    for h in range(F1 // 128):
                        ps = pc.tile([128, 128], F32, space="PSUM", tag="xr1p")
                        for ki, ks in enumerate(kt_sizes):
                            nc.tensor.matmul(
                                out=ps[:],
                                lhsT=wr1s[ki][:, h * 128:(h + 1) * 128],
                                rhs=xTo_sb[ki][:, g * 128:(g + 1) * 128],
                                start=(ki == 0), stop=False)
                        nc.tensor.matmul(out=ps[:],
                                         lhsT=bias12[:, h * 128:(h + 1) * 128],
                                         rhs=ones_row[:],
                                         start=False, stop=True)
                        t = rpool.tile([128, 128], BF, tag=f"xr1T_{g}_{h}")
                        if (g + h) % 2 == 0:
                            nc.vector.tensor_copy(out=t[:], in_=ps[:])
                        else:
                            nc.scalar.copy(out=t[:], in_=ps[:])
                        halves.append(t)
                    xr1T.append(halves)

            # ---------- phase A3: xl1_full mirror table ----------
            # batches of 4 tiles (512 rows); mirror dest row = v ^ MIR
            with tc.tile_pool(name="pD", bufs=3) as pd, \
                 tc.tile_pool(name="pDp", bufs=2, space="PSUM") as pdp:
                t = 0
                while t < n_xl1_tiles:
                    nt = min(4, n_xl1_tiles - t)
                    # don't straddle the mirror boundary
                    if t * 128 < MIR < (t + nt) * 128:
                        nt = (MIR - t * 128) // 128
                    lhs = []
                    for ki, ks in enumerate(kt_sizes):
                        lt = pd.tile([ks, 4 * 128], BF, tag=f"xl1l{ki}")
                        nc.sync.dma_start(
                            out=lt[:, :nt * 128],
                            in_=din["xT"].ap()[ki * 128:ki * 128 + ks,
                                               t * 128:(t + nt) * 128])
                        lhs.append(lt)
                    ob = pd.tile([128, 4, F1], BF, tag="xl1o")
                    for j in range(nt):
                        ps = pdp.tile([128, F1], F32, space="PSUM", tag="xl1p")
                        for ki, ks in enumerate(kt_sizes):
                            nc.tensor.matmul(
                                out=ps[:],
                                lhsT=lhs[ki][:, j * 128:(j + 1) * 128],
                                rhs=wl1s[ki][:],
                                start=(ki == 0), stop=(ki == len(kt_sizes) - 1))
                        if j % 2 == 0:
                            nc.vector.tensor_copy(out=ob[:, j, :], in_=ps[:])
                        else:
                            nc.scalar.copy(out=ob[:, j, :], in_=ps[:])
                    base = t * 128
                    mbase = base ^ MIR
                    nc.sync.dma_start(
                        out=xl1_full.ap()[mbase:mbase + nt * 128, :]
                        .rearrange("(k p) f -> p k f", k=nt),
                        in_=ob[:, :nt, :])
                    t += nt

            # ---------- edge phase helper ----------
            def edge_phase(F, xfull, esrc_sb, ah_sb, eh_mats, meanm, shift_ap,
                           bias_col, out_cb, layer):
                nhalf = F // 128
                # compute batches: (s0, b, g, noff) for each CSUB-subtile batch
                batches = []
                for s0 in range(0, NSUB, CSUB):
                    b, nstart = subtiles[s0]
                    for s in range(s0, s0 + CSUB):
                        assert subtiles[s][0] == b
                    noff = nstart % 128
                    assert noff + CSUB * (128 // b) <= 128
                    batches.append((s0, b, nstart // 128, noff))
                with tc.tile_pool(name=f"gg{layer}", bufs=3) as ggp, \
                     tc.tile_pool(name=f"ge{layer}", bufs=4) as gp, \
                     tc.tile_pool(name=f"gz{layer}", bufs=2, space="PSUM") as gz, \
                     tc.tile_pool(name=f"gl{layer}", bufs=1, space="PSUM") as gl, \
                     tc.tile_pool(name=f"gn{layer}", bufs=2, space="PSUM") as gn, \
                     tc.tile_pool(name=f"gd{layer}", bufs=1, space="PSUM") as gd, \
                     tc.tile_pool(name=f"gs{layer}", bufs=4) as gs:
                    numT = denT = None
                    cur_g = -1

                    def epilogue(g):
                        drec = gs.tile([8, 128], F32, tag="drec")
                        nc.vector.reciprocal(drec[:], denT[:])
                        drecb = gs.tile([8, 128], BF, tag="drecb")
                        nc.vector.tensor_copy(out=drecb[:], in_=drec[:])
                        onts = []
                        for h in range(nhalf):
                            rexp = gz.tile([128, CSUB * 128], F32, space="PSUM",
                                           tag=f"zt{h}")
                            nc.tensor.matmul(out=rexp[:, :128], lhsT=eh_mats[h],
                                             rhs=drecb[:], start=True, stop=True)
                            rexpb = gs.tile([128, 128], BF, tag=f"rexpb{h}",
                                            name=f"rexpb{h}")
                            nc.scalar.copy(out=rexpb[:], in_=rexp[:, :128])
                            ont = gs.tile([128, 128], BF, tag=f"ont{h}",
                                          name=f"ont{h}")
                            nc.vector.tensor_tensor(
                                out=ont[:], in0=numT[:, h * 128:(h + 1) * 128],
                                in1=rexpb[:], op=mybir.AluOpType.mult)
                            onts.append(ont)
                        cdim = C1 if layer == 1 else C2
                        ot = gl.tile([cdim, 128], F32, space="PSUM", tag="lg")
                        for h in range(nhalf):
                            nc.tensor.matmul(out=ot[:], lhsT=meanm[:, :cdim],
                                             rhs=onts[h][:], start=(h == 0),
                                             stop=(h == nhalf - 1))
                        out_cb(g, ot, bias_col)

                    xg = None
                    for bi, (s0, b, g, noff) in enumerate(batches):
                        nps = 128 // b
                        if g != cur_g:
                            if cur_g >= 0:
                                epilogue(cur_g)
                            numT = gn.tile([128, nhalf * 128], F32,
                                           space="PSUM", tag="numT")
                            denT = gd.tile([8, 128], F32, space="PSUM",
                                           tag="denT")
                            cur_g = g
                        # gather every GSUB subtiles
                        if s0 % GSUB == 0:
                            gbi = s0 // GSUB
                            xg = ggp.tile([128, GSUB, F], BF, tag="xg")
                            nc.gpsimd.dma_gather(
                                out_ap=xg[:], in_ap=xfull.ap()[MIR:, :],
                                idxs_ap=esrc_sb[:, gbi * GSUB * 8:
                                                (gbi + 1) * GSUB * 8],
                                num_idxs=GSUB * 128, num_idxs_reg=GSUB * 128,
                                elem_size=F)
                        co = (s0 % GSUB)        # subtile offset in gather tile
                        xr_h = xr1T[g] if layer == 1 else [xr2T[g]]
                        zts = []
                        for h in range(nhalf):
                            zth = gz.tile([128, CSUB * 128], F32, space="PSUM",
                                          tag=f"zt{h}", name=f"zt{h}")
                            # batched xr broadcast-add first (clears bank)
                            rep = xr_h[h][:, noff:noff + CSUB * nps, None] \
                                .broadcast_to([128, CSUB * nps, b])
                            nc.tensor.matmul(out=zth[:], lhsT=ident[:],
                                             rhs=rep, start=True, stop=False)
                            # per-subtile transpose accumulate
                            for si in range(CSUB):
                                nc.tensor.matmul(
                                    out=zth[:, si * 128:(si + 1) * 128],
                                    lhsT=xg[:, co + si,
                                            h * 128:(h + 1) * 128],
                                    rhs=ident[:],
                                    start=False, stop=(si == CSUB - 1))
                            zts.append(zth)
                        es = []
                        for h in range(nhalf):
                            # lrelu(z) = z + relu(-0.8 z)
                            r8 = gs.tile([128, CSUB * 128], BF, tag=f"r8{h}",
                                         name=f"r8{h}")
                            nc.scalar.activation(
                                out=r8[:], in_=zts[h][:],
                                func=mybir.ActivationFunctionType.Relu,
                                scale=nslope[:, :1])
                            e_sb = gs.tile([128, CSUB * 128], BF, tag=f"es{h}",
                                           name=f"es{h}")
                            nc.vector.tensor_tensor(
                                out=e_sb[:], in0=zts[h][:], in1=r8[:],
                                op=mybir.AluOpType.add)
                            es.append(e_sb)
                        lg = gl.tile([128, CSUB * 8], F32, space="PSUM",
                                     tag="lg")
                        for si in range(CSUB):
                            for h in range(nhalf):
                                nc.tensor.matmul(
                                    out=lg[:, si * 8:(si + 1) * 8],
                                    lhsT=es[h][:, si * 128:(si + 1) * 128],
                                    rhs=ah_sb[h][:],
                                    start=(h == 0), stop=(h == nhalf - 1))
                        w4 = gs.tile([128, CSUB * 8], BF, tag="w4")
                        nc.scalar.activation(
                            out=w4[:], in_=lg[:],
                            func=mybir.ActivationFunctionType.Exp,
                            bias=shift_ap[:, :1])
                        # batched one-hot S [128, CSUB, nps]
                        S4 = gs.tile([128, CSUB, nps], BF, tag="S4")
                        nc.vector.tensor_tensor(
                            out=S4[:],
                            in0=io_b[b][:, None, :]
                            .broadcast_to([128, CSUB, nps]),
                            in1=edsts_sb[:, s0:s0 + CSUB, None]
                            .broadcast_to([128, CSUB, nps]),
                            op=mybir.AluOpType.is_equal)
                        # batched y = xls * w4
                        y4 = gp.tile([128, CSUB, F], BF, tag="y4")
                        nc.vector.tensor_tensor(
                            out=y4[:].rearrange("p s (a c) -> p s a c", a=8),
                            in0=xg[:, co:co + CSUB, :]
                            .rearrange("p s (a c) -> p s a c", a=8),
                            in1=w4[:].rearrange("p (s a) -> p s a", s=CSUB)
                            [:, :, :, None]
                            .broadcast_to([128, CSUB, 8, F // 8]),
                            op=mybir.AluOpType.mult)
                        for si in range(CSUB):
                            pos = noff + si * nps
                            for h in range(nhalf):
                                nc.tensor.matmul(
                                    out=numT[:, h * 128 + pos:
                                             h * 128 + pos + nps],
                                    lhsT=y4[:, si, h * 128:(h + 1) * 128],
                                    rhs=S4[:, si, :], start=True, stop=True)
                            nc.tensor.matmul(
                                out=denT[0:8, pos:pos + nps],
                                lhsT=w4[:, si * 8:(si + 1) * 8],
                                rhs=S4[:, si, :], start=True, stop=True)
                    epilogue(cur_g)

            # ---------- phase B: layer-1 edges -> h1T ----------
            h1T = rpool.tile([HID, NODES_PAD], BF)
            oB = ctx.enter_context(tc.tile_pool(name="oB", bufs=2))

            def l1_out(g, ot_psum, bias_col):
                hrel = oB.tile([HID, 128], BF, tag="hrel")
                nc.scalar.activation(out=hrel[:], in_=ot_psum[:],
                                     func=mybir.ActivationFunctionType.Relu,
                                     bias=bias_col[:, :1])
                nc.vector.tensor_tensor(out=h1T[:, g * 128:(g + 1) * 128],
                                        in0=hrel[:],
                                        in1=mask32_sb[:, g * 128:(g + 1) * 128],
                                        op=mybir.AluOpType.mult)

            edge_phase(F1, xl1_full, esrc1_sb, a1_sb, e1h, mean1, msh1,
                       bias1t, l1_out, layer=1)

            # ---------- phase C: AllGather h1T + BN2 + xl2 + xr2T ----------
            with tc.tile_pool(name="pE", bufs=2) as pe:
                st2 = pe.tile([HID, 2], F32, tag="st2")
                nc.vector.tensor_reduce(out=st2[:, 0:1], in_=h1T[:],
                                        axis=mybir.AxisListType.X,
                                        op=mybir.AluOpType.add)
                scr2 = pe.tile([HID, NODES_PAD], BF, tag="scr2")
                nc.scalar.activation(out=scr2[:], in_=h1T[:],
                                     func=mybir.ActivationFunctionType.Square,
                                     accum_out=st2[:, 1:2])
                nc.sync.dma_start(out=ag_in.ap()[0:HID, :], in_=h1T[:])
                # stats rows (f32 bitcast into bf16 row space)
                nc.sync.dma_start(out=ag_in.ap()[HID:HID + 1, 0:2 * HID],
                                  in_=st2[:, 0:1].bitcast(BF))
                nc.sync.dma_start(out=ag_in.ap()[HID + 1:HID + 2, 0:2 * HID],
                                  in_=st2[:, 1:2].bitcast(BF))
            nc.gpsimd.collective_compute(
                "AllGather", mybir.AluOpType.bypass,
                ins=[ag_in.ap()], outs=[ag_out.ap()],
                replica_groups=[list(range(NCORES))])

            with tc.tile_pool(name="pF", bufs=1) as pf:
                # gather stat rows: each row HID f32 (=2*HID bf16)
                s2sum = pf.tile([HID, NCORES], F32, tag="s2sum")
                s2sq = pf.tile([HID, NCORES], F32, tag="s2sq")
                agf = ag_out.ap().bitcast(F32)  # [272, NODES_PAD//2]
                for c in range(NCORES):
                    r = c * (HID + 2) + HID
                    nc.sync.dma_start(out=s2sum[:, c:c + 1],
                                      in_=agf[r:r + 1, 0:HID])
                    nc.sync.dma_start(out=s2sq[:, c:c + 1],
                                      in_=agf[r + 1:r + 2, 0:HID])
                stg = pf.tile([HID, 2], F32, tag="stg2")
                nc.vector.tensor_reduce(out=stg[:, 0:1], in_=s2sum[:],
                                        axis=mybir.AxisListType.X,
                                        op=mybir.AluOpType.add)
                nc.vector.tensor_reduce(out=stg[:, 1:2], in_=s2sq[:],
                                        axis=mybir.AxisListType.X,
                                        op=mybir.AluOpType.add)
                gb = pf.tile([HID, 2], F32, tag="gb2")
                nc.sync.dma_start(out=gb[:], in_=din["gb2"].ap())
                mean = pf.tile([HID, 1], F32, tag="mean2")
                nc.vector.tensor_scalar(out=mean[:], in0=stg[:, 0:1],
                                        scalar1=RECIP_N, scalar2=None,
                                        op0=mybir.AluOpType.mult)
                q = pf.tile([HID, 1], F32, tag="q2")
                nc.vector.tensor_scalar(out=q[:], in0=stg[:, 1:2],
                                        scalar1=RECIP_N, scalar2=None,
                                        op0=mybir.AluOpType.mult)
                m2 = pf.tile([HID, 1], F32, tag="m22")
                nc.vector.tensor_tensor(out=m2[:], in0=mean[:], in1=mean[:],
                                        op=mybir.AluOpType.mult)
                var = pf.tile([HID, 1], F32, tag="var2")
                nc.vector.tensor_tensor(out=var[:], in0=q[:], in1=m2[:],
                                        op=mybir.AluOpType.subtract)
                sd = pf.tile([HID, 1], F32, tag="sd2")
                nc.scalar.activation(out=sd[:], in_=var[:],
                                     func=mybir.ActivationFunctionType.Sqrt,
                                     bias=epsb[:HID, :1])
                rstd = pf.tile([HID, 1], F32, tag="rstd2")
                nc.vector.reciprocal(rstd[:], sd[:])
                s2 = pf.tile([HID, 1], F32, tag="s2")
                nc.vector.tensor_tensor(out=s2[:], in0=gb[:, 0:1], in1=rstd[:],
                                        op=mybir.AluOpType.mult)
                ms = pf.tile([HID, 1], F32, tag="ms2")
                nc.vector.tensor_tensor(out=ms[:], in0=mean[:], in1=s2[:],
                                        op=mybir.AluOpType.mult)
                t2 = pf.tile([HID, 1], F32, tag="t2")
                nc.vector.tensor_tensor(out=t2[:], in0=gb[:, 1:2], in1=ms[:],
                                        op=mybir.AluOpType.subtract)
                # scaled weights
                wl2f = pf.tile([HID, F2], F32, tag="wl2f")
                nc.sync.dma_start(out=wl2f[:], in_=din["Wl2"].ap())
                wr2f = pf.tile([HID, F2], F32, tag="wr2f")
                nc.sync.dma_start(out=wr2f[:], in_=din["Wr2"].ap())
                wl2s = rpool.tile([HID, F2], BF)
                nc.vector.tensor_scalar(out=wl2s[:], in0=wl2f[:],
                                        scalar1=s2[:, :1], scalar2=None,
                                        op0=mybir.AluOpType.mult)
                wr2s = rpool.tile([HID, F2], BF)
                nc.vector.tensor_scalar(out=wr2s[:], in0=wr2f[:],
                                        scalar1=s2[:, :1], scalar2=None,
                                        op0=mybir.AluOpType.mult)
                t2b = pf.tile([HID, 1], BF, tag="t2b")
                nc.vector.tensor_copy(out=t2b[:], in_=t2[:])
                wsum = pf.tile([HID, F2], BF, tag="wsum")
                nc.vector.tensor_tensor(out=wsum[:], in0=wl2f[:], in1=wr2f[:],
                                        op=mybir.AluOpType.add)
                with tc.tile_pool(name="pFp", bufs=1, space="PSUM") as pfp:
                    bps = pfp.tile([1, F2], F32, space="PSUM")
                    nc.tensor.matmul(out=bps[:], lhsT=t2b[:], rhs=wsum[:],
                                     start=True, stop=True)
                    bias22 = rpool.tile([1, F2], BF)
                    nc.vector.tensor_copy(out=bias22[:], in_=bps[:])
                with tc.tile_pool(name="pFq", bufs=1, space="PSUM") as pfq:
                    wmf2 = pf.tile([HID, OUT], F32, tag="wmf2")
                    nc.sync.dma_start(out=wmf2[:], in_=din["Wl2m"].ap())
                    wmb2 = pf.tile([HID, OUT], BF, tag="wmb2")
                    nc.vector.tensor_copy(out=wmb2[:], in_=wmf2[:])
                    blp2 = pfq.tile([1, OUT], F32, space="PSUM")
                    nc.tensor.matmul(out=blp2[:], lhsT=t2b[:], rhs=wmb2[:],
                                     start=True, stop=True)
                    blr2 = pf.tile([1, OUT], F32, tag="blr2")
                    nc.vector.tensor_copy(out=blr2[:], in_=blp2[:])
                    nc.sync.dma_start(out=bl2_d.ap(), in_=blr2[:])
                blc2 = rpool.tile([OUT, 1], F32)
                nc.sync.dma_start(out=blc2[:], in_=bl2_d.ap())
                bias2t = rpool.tile([OUT, 1], F32)
                nc.vector.tensor_tensor(out=bias2t[:], in0=blc2[:], in1=b2c_sb[:],
                                        op=mybir.AluOpType.add)

            # xl2_full mirror table (batches of 4 column-tiles per c block)
            with tc.tile_pool(name="pG", bufs=3) as pg, \
                 tc.tile_pool(name="pGp", bufs=2, space="PSUM") as pgp:
                for c_src in range(NCORES):
                    off = 0
                    while off < NODES_PAD:
                        nt = min(4, (NODES_PAD - off) // 128)
                        base = c_src * NODES_PAD + off
                        # don't straddle the mirror boundary
                        if base < MIR < base + nt * 128:
                            nt = (MIR - base) // 128
                        lhs2 = pg.tile([HID, 4 * 128], BF, tag="xl2l")
                        nc.sync.dma_start(
                            out=lhs2[:, :nt * 128],
                            in_=ag_out.ap()[c_src * (HID + 2):
                                            c_src * (HID + 2) + HID,
                                            off:off + nt * 128])
                        ob = pg.tile([128, 4, F2], BF, tag="xl2o")
                        for j in range(nt):
                            ps = pgp.tile([128, F2], F32, space="PSUM",
                                          tag="xl2p")
                            nc.tensor.matmul(
                                out=ps[:],
                                lhsT=lhs2[:, j * 128:(j + 1) * 128],
                                rhs=wl2s[:], start=True, stop=True)
                            if j % 2 == 0:
                                nc.vector.tensor_copy(out=ob[:, j, :], in_=ps[:])
                            else:
                                nc.scalar.copy(out=ob[:, j, :], in_=ps[:])
                        mbase = base ^ MIR
                        nc.sync.dma_start(
                            out=xl2_full.ap()[mbase:mbase + nt * 128, :]
                            .rearrange("(k p) f -> p k f", k=nt),
                            in_=ob[:, :nt, :])
                        off += nt * 128
            # xr2T resident
            xr2T = []
            with tc.tile_pool(name="pH", bufs=2, space="PSUM") as ph:
                for g in range(NG):
                    ps = ph.tile([128, 128], F32, space="PSUM", tag="xr2p")
                    nc.tensor.matmul(out=ps[:], lhsT=wr2s[:],
                                     rhs=h1T[:, g * 128:(g + 1) * 128],
                                     start=True, stop=False)
                    nc.tensor.matmul(out=ps[:], lhsT=bias22[:], rhs=ones_row[:],
                                     start=False, stop=True)
                    t = rpool.tile([128, 128], BF, tag=f"xr2T_{g}")
                    if g % 2 == 0:
                        nc.vector.tensor_copy(out=t[:], in_=ps[:])
                    else:
                        nc.scalar.copy(out=t[:], in_=ps[:])
                    xr2T.append(t)

            # ---------- phase D: layer-2 edges -> outT ----------
            oD = ctx.enter_context(tc.tile_pool(name="oD", bufs=2))

            def l2_out(g, ot_psum, bias_col):
                ob = oD.tile([OUT, 128], F32, tag="ob")
                nc.scalar.activation(out=ob[:], in_=ot_psum[:],
                                     func=mybir.ActivationFunctionType.Identity,
                                     bias=bias_col[:, :1])
                nc.sync.dma_start(out=outT.ap()[:, g * 128:(g + 1) * 128],
                                  in_=ob[:])

            edge_phase(F2, xl2_full, esrc2_sb, a2_sb, e2h, mean2, msh2,
                       bias2t, l2_out, layer=2)

    nc.compile()
    return nc


_CACHE = {}


def _get_nc(cfg, meta):
    key = (cfg.N, cfg.IN, cfg.HID, cfg.OUT, meta["NSUB"], meta["NODES_PAD"])
    if key not in _CACHE:
        _CACHE[key] = _build(cfg, meta)
    return _CACHE[key]


def run(cfg, inputs):
    x = np.asarray(inputs["x"], np.float32)
    ei = np.asarray(inputs["edge_index"], np.int32)
    W = {k: np.asarray(inputs[k], np.float32) for k in
         ("Wl1", "Wr1", "att1", "b1", "gamma1", "beta1",
          "Wl2", "Wr2", "att2", "b2", "gamma2", "beta2")}
    meta = _preprocess(cfg, x, ei, W)
    nc = _get_nc(cfg, meta)
    res = run_bass_kernel_spmd(nc, meta["in_maps"], core_ids=list(range(NCORES)))
    out = np.empty((cfg.N, cfg.OUT), np.float32)
    proc = meta["proc"]
    for c in range(NCORES):
        oT = res.results[c]["outT"]      # [OUT, NODES_PAD]
        sel = proc[c] >= 0
        out[c * cfg.NL + proc[c][sel]] = oT[:, sel].T
    return out, meta, nc


def kernel(**inputs):
    cfg = Cfg(50000, 200, 32, 16, m1=8.0, m2=10.0)
    out, _, _ = run(cfg, inputs)
    return out
